# revision 1
# baseline (speedup 1.0000x reference)
"""Multi-head attention with relative position bias (music-transformer skew)
on 8 Trainium2 NeuronCores.

Sharding: batch x head-group. Core c handles batch b = c//4 and heads
4*(c%4) .. 4*(c%4)+3 (tensor-parallel split of the 16 heads / W column dims,
Wo row-parallel). Partial outputs are summed on the host (the all-reduce
equivalent), bias added on the host.

Per-core pipeline (bf16 matmuls, fp32 PSUM accumulation):
  x^T via PE transposes -> Q^T,K^T ([col,n]) and V ([m,col]) projections.
  Per head pair, one software-pipelined loop over row blocks i:
    expR = exp((Q/8) @ E1^T) -> DRAM (exp fused into the mandatory
    PSUM->SBUF copy), skew-read back (stride N-1 access pattern) as relE
    with the causal diagonal pre-masked by a 0/1 triangle;
    expS = exp((Q/8) @ K^T) straight out of PSUM;
    P = expS * relE with row sums Z fused into the same DVE op;
    P^T = P.T @ diag(1/Z) on the PE (normalization for free), delayed two
    iterations so the PE never stalls on the softmax chain;
  then O^T accumulates V-side (two heads in complementary PE column groups)
  and y = O^T.T @ Wo.  The two heads of a pair interleave their K=64
  matmuls in complementary PE row groups.
"""

import numpy as np

import concourse.bass as bass
import concourse.tile as tile
from concourse import bacc, mybir
from concourse.bass import ds, ts
from concourse.bass_utils import run_bass_kernel_spmd
from concourse.masks import make_identity

B, N, D, H, DH, DO = 2, 1024, 1024, 16, 64, 1024
HPC = 4              # heads per core
COLS = HPC * DH      # 256 projection columns per core
NB = N // 128        # 8 row blocks
KT = D // 128        # 8 contraction tiles
F32 = mybir.dt.float32
BF16 = mybir.dt.bfloat16
SCALE = 1.0 / np.sqrt(DH)
EXP = mybir.ActivationFunctionType.Exp
MULT = mybir.AluOpType.mult


def _body(tc):
    nc = tc.nc
    xb = nc.dram_tensor("xb", [N, D], F32, kind="ExternalInput")
    wq = nc.dram_tensor("wq", [D, COLS], F32, kind="ExternalInput")
    wk = nc.dram_tensor("wk", [D, COLS], F32, kind="ExternalInput")
    wv = nc.dram_tensor("wv", [D, COLS], F32, kind="ExternalInput")
    wo = nc.dram_tensor("wo", [COLS, DO], F32, kind="ExternalInput")
    e1 = nc.dram_tensor("e1", [N, DH], F32, kind="ExternalInput")
    qes = [nc.dram_tensor(f"qes{h}", [N, N], BF16) for h in range(HPC)]

    from contextlib import ExitStack
    ctx = ExitStack()
    singles = ctx.enter_context(tc.tile_pool(name="singles", bufs=1))
    persist = ctx.enter_context(tc.tile_pool(name="persist", bufs=1))
    stage = ctx.enter_context(tc.tile_pool(name="stage", bufs=2))
    work = ctx.enter_context(tc.tile_pool(name="work", bufs=2))
    ptp = ctx.enter_context(tc.tile_pool(name="ptp", bufs=1))
    pss = ctx.enter_context(tc.tile_pool(name="pss", bufs=8, space="PSUM"))

    ident = singles.tile([128, 128], BF16, tag="ident", name="ident")
    make_identity(nc, ident)
    # 0/1 lower-triangular (causal keep) mask, bf16
    tri01 = singles.tile([128, 128], BF16, tag="tri01", name="tri01")
    nc.gpsimd.memset(tri01, 1.0)
    nc.gpsimd.affine_select(
        out=tri01, in_=tri01, compare_op=mybir.AluOpType.is_ge,
        fill=0.0, base=0, pattern=[[-1, 128]], channel_multiplier=1,
    )

    # ---- x loads (DVE HWDGE queue), casts, PE transposes per half ----
    xT = [persist.tile([128, N], BF16, tag=f"xT{k}", name=f"xT{k}")
          for k in range(KT)]
    xls = []
    for half in range(2):
        xl = stage.tile([128, 4, D], F32, tag="ld", name="xld")
        nc.scalar.dma_start(
            out=xl,
            in_=xb.rearrange("(nt p) d -> p nt d", p=128)[:, ds(4 * half, 4), :],
        )
        xls.append(xl)
        if half == 0:
            wl_q = stage.tile([128, KT, COLS], F32, tag="ld", name="wldq")
            nc.scalar.dma_start(
                out=wl_q, in_=wq.rearrange("(kt p) c -> p kt c", p=128))
            el = stage.tile([128, 8, DH], F32, tag="lde", name="eld", bufs=1)
            nc.scalar.dma_start(
                out=el, in_=e1.rearrange("(nt p) d -> p nt d", p=128))

    def cast_transpose_half(half):
        xc = stage.tile([128, 4, D], BF16, tag="xc", name="xc", bufs=2)
        nc.vector.tensor_copy(xc, xls[half])
        for k in range(KT):
            ps = pss.tile([128, 512], BF16, tag="sp", name="sp")
            for q in range(4):
                nc.tensor.transpose(
                    ps[:, ts(q, 128)], xc[:, q, ts(k, 128)], ident
                )
            nc.vector.tensor_copy(xT[k][:, ds(512 * half, 512)], ps)

    cast_transpose_half(0)

    # ---- E1^T [64, 1024] bf16, duplicated to partitions 64..127 ----
    e1t = singles.tile([128, N], BF16, tag="e1t", name="e1t")
    ec = stage.tile([128, 8, DH], BF16, tag="ec", name="ec", bufs=1)
    nc.vector.tensor_copy(ec, el)
    for half in range(2):
        ps = pss.tile([64, 512], BF16, tag="sp", name="sp")
        for q in range(4):
            nc.tensor.transpose(ps[:, ts(q, 128)], ec[:, 4 * half + q, :], ident)
        nc.vector.tensor_copy(e1t[0:64, ds(512 * half, 512)], ps)
    nc.scalar.dma_start(out=e1t[64:128, :], in_=e1t[0:64, :])

    wq_bf = persist.tile([128, KT, COLS], BF16, tag="wqb", name="wqb")
    nc.vector.tensor_copy(wq_bf, wl_q)

    qt = [persist.tile([128, N], BF16, tag=f"qt{c}", name=f"qt{c}") for c in range(2)]
    kt_sb = [persist.tile([128, N], BF16, tag=f"kt{c}", name=f"kt{c}") for c in range(2)]

    def emit_proj(w_all, dest, ct, nh, scale):
        ps = pss.tile([128, 512], F32, tag="sp", name="sp")
        for k in range(KT):
            nc.tensor.matmul(
                ps, w_all[:, k, ts(ct, 128)], xT[k][:, ts(nh, 512)],
                start=(k == 0), stop=(k == KT - 1),
            )
        if scale == 1.0:
            nc.vector.tensor_copy(dest[ct][:, ts(nh, 512)], ps)
        else:
            nc.vector.tensor_scalar_mul(dest[ct][:, ts(nh, 512)], ps, scale)

    emit_proj(wq_bf, qt, 0, 0, SCALE)
    # expR = exp((Q/8) @ E1^T) -> DRAM scratch; exp fused into the mandatory
    # PSUM->SBUF copy.  For i<4 only the causally-needed column window
    # [896-128i, 1024) plus the wrap chunk [0, 128) is computed/written.
    def emit_expr(pr, i0=0, i1=NB):
        lhs = (2 * pr, 2 * pr + 1)
        qt_t = qt[pr]
        for i in range(i0, i1):
            if i < 3:
                # main window [896-128i, 1024) and wrap chunk [0, 128) share
                # one PSUM bank and one fused exp
                lo = 896 - 128 * i
                mw = 128 * (i + 1)
                tw = mw + 128
                pps = [pss.tile([128, 512], F32, tag="sp", name="sp")
                       for _ in range(2)]
                for hs in range(2):
                    base = 64 * hs
                    nc.tensor.matmul(
                        pps[hs][:, :mw],
                        qt_t[base:base + 64, ts(i, 128)],
                        e1t[base:base + 64, ds(lo, mw)],
                        start=True, stop=True,
                    )
                    nc.tensor.matmul(
                        pps[hs][:, ds(mw, 128)],
                        qt_t[base:base + 64, ts(i, 128)],
                        e1t[base:base + 64, ds(0, 128)],
                        start=True, stop=True, skip_group_check=True,
                    )
                for hs in range(2):
                    qec = work.tile([128, 1024], BF16, tag=f"qec{hs}",
                                    name="qec", bufs=2)
                    nc.scalar.activation(qec[:, :tw], pps[hs][:, :tw], EXP)
                    nc.sync.dma_start(
                        out=qes[lhs[hs]][ts(i, 128), ds(lo, mw)],
                        in_=qec[:, :mw])
                    nc.sync.dma_start(
                        out=qes[lhs[hs]][ts(i, 128), ds(0, 128)],
                        in_=qec[:, ds(mw, 128)])
            elif i == 3:
                lo = 512
                mw = 512
                pps = [[pss.tile([128, 512], F32, tag="sp", name="sp")
                        for _ in range(2)] for _ in range(2)]
                for hs in range(2):
                    base = 64 * hs
                    nc.tensor.matmul(
                        pps[hs][0],
                        qt_t[base:base + 64, ts(i, 128)],
                        e1t[base:base + 64, ds(512, 512)],
                        start=True, stop=True,
                    )
                    nc.tensor.matmul(
                        pps[hs][1][:, :128],
                        qt_t[base:base + 64, ts(i, 128)],
                        e1t[base:base + 64, ds(0, 128)],
                        start=True, stop=True,
                    )
                for hs in range(2):
                    qec = work.tile([128, 1024], BF16, tag=f"qec{hs}",
                                    name="qec", bufs=2)
                    nc.scalar.activation(qec[:, :512], pps[hs][0], EXP)
                    nc.scalar.activation(
                        qec[:, ds(512, 128)], pps[hs][1][:, :128], EXP)
                    nc.sync.dma_start(
                        out=qes[lhs[hs]][ts(i, 128), ds(512, 512)],
                        in_=qec[:, :512])
                    nc.sync.dma_start(
                        out=qes[lhs[hs]][ts(i, 128), ds(0, 128)],
                        in_=qec[:, ds(512, 128)])
            else:
                pps = [[pss.tile([128, 512], F32, tag="sp", name="sp")
                        for _ in range(2)] for _ in range(2)]
                for h2 in range(2):
                    for hs in range(2):
                        base = 64 * hs
                        nc.tensor.matmul(
                            pps[hs][h2],
                            qt_t[base:base + 64, ts(i, 128)],
                            e1t[base:base + 64, ts(h2, 512)],
                            start=True, stop=True,
                        )
                for hs in range(2):
                    qec = work.tile([128, 1024], BF16, tag=f"qec{hs}",
                                    name="qec", bufs=2)
                    for h2 in range(2):
                        nc.scalar.activation(qec[:, ts(h2, 512)], pps[hs][h2], EXP)
                        nc.sync.dma_start(
                            out=qes[lhs[hs]][ts(i, 128), ts(h2, 512)],
                            in_=qec[:, ts(h2, 512)])


    emit_expr(0, 0, 4)
    cast_transpose_half(1)
    emit_proj(wq_bf, qt, 0, 1, SCALE)
    emit_expr(0, 4, 8)

    # K / V weight loads + casts (DVE queue), remaining projections
    wl_k = stage.tile([128, KT, COLS], F32, tag="ld", name="wldk")
    nc.scalar.dma_start(out=wl_k, in_=wk.rearrange("(kt p) c -> p kt c", p=128))
    wk_bf = persist.tile([128, KT, COLS], BF16, tag="wkb", name="wkb")
    nc.vector.tensor_copy(wk_bf, wl_k)
    wl_v = stage.tile([128, KT, COLS], F32, tag="ld", name="wldv")
    nc.scalar.dma_start(out=wl_v, in_=wv.rearrange("(kt p) c -> p kt c", p=128))
    wv_bf = persist.tile([128, KT, COLS], BF16, tag="wvb", name="wvb")
    nc.vector.tensor_copy(wv_bf, wl_v)

    emit_proj(wk_bf, kt_sb, 0, 0, 1.0)
    emit_proj(wk_bf, kt_sb, 0, 1, 1.0)
    emit_proj(wq_bf, qt, 1, 0, SCALE)
    emit_proj(wq_bf, qt, 1, 1, SCALE)
    emit_proj(wk_bf, kt_sb, 1, 0, 1.0)
    emit_proj(wk_bf, kt_sb, 1, 1, 1.0)

    # V: [m, col] -- one [128, 8, 256] bf16 tile; quarters emitted inside
    # pair 0's S loop to fill the PE while the softmax chain runs
    v_sb = persist.tile([128, NB, COLS], BF16, tag="vsb", name="vsb")

    def emit_v(mp):
        ps = pss.tile([128, 512], F32, tag="sp", name="sp")
        for sub in range(2):
            mb = 2 * mp + sub
            for k in range(KT):
                nc.tensor.matmul(
                    ps[:, ds(256 * sub, 256)],
                    xT[k][:, ts(mb, 128)], wv_bf[:, k, :],
                    start=(k == 0), stop=(k == KT - 1),
                )
        nc.vector.tensor_copy(
            v_sb[:, ds(2 * mp, 2), :].rearrange("p a b -> p (a b)"), ps
        )

    wo_bf = persist.tile([128, 2, DO], BF16, tag="wob", name="wob")
    ot = [persist.tile([128, N], BF16, tag=f"ot{c}", name=f"ot{c}") for c in range(2)]
    ys = [nc.dram_tensor(f"y{pr}", [N, DO], F32, kind="ExternalOutput")
          for pr in range(2)]

    # ---- per-pair attention, with cross-pair interleaving ----
    pts_all = {}
    state = {}

    def emit_pt(pr, i):
        """P^T = P.T @ diag(1/Z) for row block i (delayed 2 iterations)."""
        pts = pts_all[pr]
        for hs in range(2):
            p_sb, dg = state.pop((pr, i, hs))
            for g in range(2):
                jn = min(i + 1 - 4 * g, 4)
                if jn <= 0:
                    break
                ps2 = pss.tile([128, 512], F32, tag="sp", name="sp")
                for jj in range(jn):
                    nc.tensor.matmul(
                        ps2[:, ts(jj, 128)],
                        p_sb[:, ts(4 * g + jj, 128)], dg,
                        start=True, stop=True,
                    )
                nc.vector.tensor_copy(
                    pts[hs][g][:, 0:jn, ts(i, 128)],
                    ps2[:, ds(0, 128 * jn)].rearrange("p (a b) -> p a b", a=jn),
                )

    def s_iter(pr, i):
        """One row block of the S / expS / P / 1/Z chain for pair pr."""
        lhs = (2 * pr, 2 * pr + 1)
        qt_t, kt_t = qt[pr], kt_sb[pr]
        width = 128 * (i + 1)
        nch = (width + 511) // 512

        rels = []
        for hs in range(2):
            rel = work.tile([128, 1024], BF16, tag=f"rel{hs}",
                            name="rel", bufs=3)
            nc.gpsimd.dma_start(out=rel[:, :width], in_=bass.AP(
                tensor=qes[lhs[hs]][:, :].tensor,
                offset=(N - 1) * (128 * i + 1),
                ap=[[N - 1, 128], [1, width]],
            ))
            nc.gpsimd.tensor_tensor(
                out=rel[:, ds(width - 128, 128)],
                in0=rel[:, ds(width - 128, 128)], in1=tri01, op=MULT,
            )
            rels.append(rel)

        spp = [[pss.tile([128, 512], F32, tag="sp", name="sp")
                for _ in range(nch)] for _ in range(2)]
        for c in range(nch):
            cw = min(512, width - 512 * c)
            for hs in range(2):
                base = 64 * hs
                nc.tensor.matmul(
                    spp[hs][c][:, :cw],
                    qt_t[base:base + 64, ts(i, 128)],
                    kt_t[base:base + 64, ds(512 * c, cw)],
                    start=True, stop=True,
                )
        for hs in range(2):
            es = work.tile([128, 1024], BF16, tag=f"es{hs}", name="es", bufs=2)
            for c in range(nch):
                cw = min(512, width - 512 * c)
                nc.scalar.activation(
                    es[:, ds(512 * c, cw)], spp[hs][c][:, :cw], EXP
                )
            p_sb = work.tile([128, 1024], BF16, tag=f"p{hs}", name="p", bufs=5)
            z = work.tile([128, 1], F32, tag=f"z{hs}", name="z")
            nc.vector.scalar_tensor_tensor(
                out=p_sb[:, :width], in0=es[:, :width], scalar=1.0,
                in1=rels[hs][:, :width], op0=MULT, op1=MULT, accum_out=z,
            )
            r = work.tile([128, 1], F32, tag=f"r{hs}", name="r")
            nc.vector.reciprocal(r, z)
            dg = work.tile([128, 128], BF16, tag=f"dg{hs}", name="dg", bufs=5)
            nc.vector.tensor_scalar_mul(dg, ident, r)
            state[(pr, i, hs)] = (p_sb, dg)

        if i >= 3:
            emit_pt(pr, i - 3)
        if i == NB - 1:
            for ii in (NB - 3, NB - 2, NB - 1):
                emit_pt(pr, ii)

    pv_pos = {}

    def emit_pv(pr, ig, iis=(0, 1, 2, 3)):
        """O^T accumulation for row blocks 4*ig+iis of pair pr."""
        lhs = (2 * pr, 2 * pr + 1)
        pts = pts_all[pr]
        if (pr, ig) not in pv_pos:
            pv_pos[(pr, ig)] = [
                pss.tile([128, 512], F32, tag="sp", name="sp")
                for _ in range(2)]
        pos = pv_pos[(pr, ig)]
        for ii in iis:
            i = 4 * ig + ii
            for j in range(i + 1):
                for hs in range(2):
                    base = 64 * hs
                    nc.tensor.matmul(
                        pos[hs][base:base + 64, ts(ii, 128)],
                        v_sb[:, j, ds(64 * lhs[hs], 64)],
                        pts[hs][j // 4][:, j % 4, ts(i, 128)],
                        start=(j == 0), stop=(j == i),
                        tile_position=(0, base),
                    )
        if iis[-1] != 3:
            return
        for hs in range(2):
            base = 64 * hs
            if pr == 0:
                nc.scalar.copy(
                    ot[pr][base:base + 64, ds(512 * ig, 512)],
                    pos[hs][base:base + 64, :],
                )
            else:
                nc.vector.tensor_copy(
                    ot[pr][base:base + 64, ds(512 * ig, 512)],
                    pos[hs][base:base + 64, :],
                )

    def emit_yproj(pr, i):
        """Partial output projection y_pr row block i."""
        ysb = work.tile([128, 1024], F32, tag=f"ypr{pr}", name="ysb", bufs=2)
        for oh in range(2):
            ps = pss.tile([128, 512], F32, tag="sp", name="sp")
            nc.tensor.matmul(
                ps, ot[pr][:, ts(i, 128)], wo_bf[:, pr, ds(512 * oh, 512)],
                start=True, stop=True,
            )
            if oh == 0:
                nc.scalar.copy(ysb[:, ts(oh, 512)], ps)
            else:
                nc.vector.tensor_copy(ysb[:, ts(oh, 512)], ps)
            if (2 * i + oh) % 2 == 0:
                nc.gpsimd.dma_start(
                    out=ys[pr][ts(i, 128), ds(512 * oh, 512)],
                    in_=ysb[:, ts(oh, 512)])
            else:
                nc.sync.dma_start(
                    out=ys[pr][ts(i, 128), ds(512 * oh, 512)],
                    in_=ysb[:, ts(oh, 512)])

    for pr in range(2):
        pts_all[pr] = [
            [ptp.tile([128, 4, 1024], BF16, tag=f"pts{hs}{g}", name="pts")
             for g in range(2)] for hs in range(2)]

        for i in range(NB):
            s_iter(pr, i)
            if pr == 0:
                # pair 1's expR rides along pair 0's softmax chain; V
                # quarters fill the PE as well
                emit_expr(1, i, i + 1)
                if i in (1, 3, 5, 7):
                    emit_v((i - 1) // 2)
            if pr == 1:
                # pair 0's PV / output projection rides along pair 1's
                # DVE/ACT-bound softmax chain
                if i == 1:
                    emit_pv(0, 0)
                elif i == 3:
                    emit_pv(0, 1)
                elif i in (4, 5, 6):
                    emit_yproj(0, 2 * (i - 4))
                    emit_yproj(0, 2 * (i - 4) + 1)
                elif i == 7:
                    emit_yproj(0, 6)
                    emit_yproj(0, 7)
                    emit_pv(1, 0)
        if pr == 0:
            wol = stage.tile([128, 2, DO], F32, tag="ld", name="wold")
            nc.scalar.dma_start(
                out=wol, in_=wo.rearrange("(ct p) c -> p ct c", p=128))
            nc.vector.tensor_copy(wo_bf, wol)

    emit_pv(1, 1)
    for i in range(NB):
        emit_yproj(1, i)

    ctx.close()


_NC_CACHE = None


def _get_nc():
    global _NC_CACHE
    if _NC_CACHE is None:
        nc = bacc.Bacc(
            "TRN2", target_bir_lowering=False, debug=False, num_devices=8
        )
        with tile.TileContext(nc) as tc:
            _body(tc)
        nc.compile()
        _NC_CACHE = nc
    return _NC_CACHE


def make_in_maps(x, E_rel, Wq, Wk, Wv, Wo):
    in_maps = []
    for c in range(8):
        b, g = c // 4, c % 4
        cols = slice(COLS * g, COLS * (g + 1))
        in_maps.append({
            "xb": np.ascontiguousarray(x[b], dtype=np.float32),
            "wq": np.ascontiguousarray(Wq[:, cols], dtype=np.float32),
            "wk": np.ascontiguousarray(Wk[:, cols], dtype=np.float32),
            "wv": np.ascontiguousarray(Wv[:, cols], dtype=np.float32),
            "wo": np.ascontiguousarray(Wo[cols, :], dtype=np.float32),
            "e1": np.ascontiguousarray(E_rel[:N], dtype=np.float32),
        })
    return in_maps


def combine(results, bo):
    parts = [
        np.asarray(results[c]["y0"], dtype=np.float32)
        + np.asarray(results[c]["y1"], dtype=np.float32)
        for c in range(8)
    ]
    out0 = parts[0] + parts[1] + parts[2] + parts[3] + bo.astype(np.float32)
    out1 = parts[4] + parts[5] + parts[6] + parts[7] + bo.astype(np.float32)
    return np.stack([out0, out1]).astype(np.float32)


def kernel(x, E_rel, mask, Wq, Wk, Wv, Wo, bo, **_):
    nc = _get_nc()
    in_maps = make_in_maps(
        np.asarray(x), np.asarray(E_rel), np.asarray(Wq), np.asarray(Wk),
        np.asarray(Wv), np.asarray(Wo),
    )
    res = run_bass_kernel_spmd(nc, in_maps, list(range(8)))
    return combine(res.results, np.asarray(bo))

